# revision 11
# baseline (speedup 1.0000x reference)
"""Trainium2 Bass kernel for the DQN GNN message-passing module.

Contract: kernel(**inputs) takes the FULL unsharded inputs (as produced by
setup_inputs) and returns the FULL [B, N] output. Internally the batch is
sharded across 8 NeuronCores (pure data parallel), a Bass/Tile kernel is
compiled and run via run_bass_kernel_spmd, and the per-core outputs are
concatenated.

Math (per batch b, N=256 nodes, D=5):
  term1 = feat @ W1.T + b1
  S[j,d] = sum_i relu(W4[d]*x[i,j] + b4[d])        (x = nodeLocations[b])
  term3 = S @ W3.T + b3
  conn  = (x > 0)
  mu_0 = 0;  mu_{t+1} = relu(A + (conn @ (mu_t @ W2.T)))   A = term1+term3+b2
  out = relu([W6(sum_i mu) | W7 mu]) @ W5.T + b5

Key tricks:
  * per-d case analysis of relu(w*x+c) over the actual input range
    [xmin,xmax]: always-nonpositive dims contribute 0, always-nonnegative
    dims are linear in sum_i x, only "knee inside range" dims need an
    elementwise pass (max(w*x,-c), the +c folded into constants).
  * all sum-over-i reductions are PE matmuls with the DATA as the
    128x128 stationary operand and a [128,1] ones moving operand, so the
    result lands with the free index on PSUM partitions.
  * conn transposed on PE (fp16), compare fused into the PSUM drain.
"""

import os
import numpy as np
from contextlib import ExitStack

N_CORES = 8
N = 256
D = 5

_CACHE = {}


def _f(x):
    return float(np.asarray(x))


def _build(B_shard, n_iters, w, xmin, xmax):
    import concourse.bass as bass
    import concourse.bacc as bacc
    import concourse.tile as tile
    import concourse.mybir as mybir

    Alu = mybir.AluOpType
    f32 = mybir.dt.float32
    f16 = mybir.dt.float16

    W1, b1 = w["W1"], w["b1"]
    W2, b2 = w["W2"], w["b2"]
    W3, b3 = w["W3"], w["b3"]
    W4, b4 = w["W4"], w["b4"]
    W5, b5 = w["W5"], w["b5"]
    W6, b6 = w["W6"], w["b6"]
    W7, b7 = w["W7"], w["b7"]

    # --- case analysis for S[j,d] = sum_i relu(w_d x + c_d), x in [xmin,xmax]
    w4 = [_f(W4[d, 0]) for d in range(D)]
    c4 = [_f(b4[d]) for d in range(D)]
    # kind per d: 'Z' (relu==0), 'L' (relu==w x+c), 'H' (needs elementwise)
    kinds = []
    for d in range(D):
        lo = w4[d] * xmin + c4[d]
        hi = w4[d] * xmax + c4[d]
        if max(lo, hi) <= 0.0:
            kinds.append("Z")
        elif min(lo, hi) >= 0.0:
            kinds.append("L")
        else:
            kinds.append("H")
    need_sx = any(k == "L" for k in kinds)
    hard_ds = [d for d in range(D) if kinds[d] == "H"]
    # S_d = w_d*SX + 256*c_d            (L)
    #     = SH_d + 256*c_d              (H), SH_d = sum_i max(w_d x, -c_d)
    #     = 0                           (Z)
    # term3[n,d'] = sum_d W3[d',d] S_d[n] + b3[d']
    # A[n,d'] = term1[n,d'] + term3[n,d'] + b2[d']
    # constant part of A (indep of data, except through S constants):
    a_const = [0.0] * D
    sx_coef = [0.0] * D  # coefficient of SX in A[., d']
    sh_coef = {h: [0.0] * D for h in hard_ds}  # coefficient of SH_h
    for dp in range(D):
        a_const[dp] = _f(b1[dp]) + _f(b2[dp]) + _f(b3[dp])
        for d in range(D):
            if kinds[d] == "L":
                a_const[dp] += _f(W3[dp, d]) * (N * c4[d])
                sx_coef[dp] += _f(W3[dp, d]) * w4[d]
            elif kinds[d] == "H":
                a_const[dp] += _f(W3[dp, d]) * (N * c4[d])
                sh_coef[d][dp] = _f(W3[dp, d])

    # S quantity slots in S_all: q=0 -> SX (if needed), then hard dims
    quants = []
    if need_sx:
        quants.append(("sx", None))
    for h in hard_ds:
        quants.append(("h", h))
    NQ = len(quants)

    nc = bacc.Bacc("TRN2", target_bir_lowering=False, debug=False)
    nl_d = nc.dram_tensor("nl", [B_shard, N, N], f32, kind="ExternalInput").ap()
    feat_d = nc.dram_tensor("feat", [B_shard, N, D], f32, kind="ExternalInput").ap()
    out_d = nc.dram_tensor("out", [B_shard, N], f32, kind="ExternalOutput").ap()

    NB = B_shard          # batches per core
    NT = N // 128         # partition tiles per node dim (2)
    FW = NB * NT          # 128 for NB=64 (b,it)-columns
    GRP = 16              # batches per PSUM S-group
    NG = (NB + GRP - 1) // GRP

    assert NB % GRP == 0 and FW <= 128

    with tile.TileContext(nc) as tc, ExitStack() as ctx:
        singles = ctx.enter_context(tc.tile_pool(name="singles", bufs=1))
        xpool = ctx.enter_context(tc.tile_pool(name="xpool", bufs=8))
        x16pool = ctx.enter_context(tc.tile_pool(name="x16pool", bufs=3))
        rpool = ctx.enter_context(tc.tile_pool(name="rpool", bufs=3))
        psT = ctx.enter_context(tc.tile_pool(name="psT", bufs=3, space="PSUM"))
        psS = ctx.enter_context(tc.tile_pool(name="psS", bufs=2, space="PSUM"))
        psMU = ctx.enter_context(tc.tile_pool(name="psMU", bufs=2, space="PSUM"))
        psE = ctx.enter_context(tc.tile_pool(name="psE", bufs=1, space="PSUM"))
        scr = ctx.enter_context(tc.tile_pool(name="scr", bufs=2))

        # ---------- constants ----------
        ident16 = singles.tile([128, 128], f16)
        ident32 = singles.tile([128, 128], f32)
        from concourse import masks as _masks
        _masks.make_identity(nc, ident16[:, :])
        _masks.make_identity(nc, ident32[:, :])
        ones16 = singles.tile([128, 1], f16)
        nc.vector.memset(ones16[:, :], 1.0)
        ones32 = singles.tile([128, 1], f32)
        nc.vector.memset(ones32[:, :], 1.0)
        # pairmask[p, c] = 1 iff p//NT == c  (shape [128, NB])  -- built via
        # affine iota: keep where 0 <= p - NT*c <= NT-1
        pairmask = singles.tile([128, NB], f32)
        nc.gpsimd.memset(pairmask[:, :], 1.0)
        nc.gpsimd.affine_select(
            out=pairmask[:, :], in_=pairmask[:, :],
            pattern=[[-NT, NB]], compare_op=Alu.is_ge, fill=0.0,
            base=0, channel_multiplier=1)
        nc.gpsimd.affine_select(
            out=pairmask[:, :], in_=pairmask[:, :],
            pattern=[[NT, NB]], compare_op=Alu.is_ge, fill=0.0,
            base=NT - 1, channel_multiplier=-1)
        # pmkT = pairmask.T  [NB, 128]
        ps_tmp = psE.tile([128, 128], f32, tag="pse")
        nc.tensor.transpose(ps_tmp[:NB, :128], pairmask[:, :NB], ident32[:, :])
        pmkT = singles.tile([NB, 128], f32)
        nc.vector.tensor_copy(pmkT[:, :], ps_tmp[:NB, :128])

        # ---------- per-core big buffers ----------
        # connT_all: [128(j_lo), (b, jt, i)] fp16
        connT = singles.tile([128, NB * NT * N], f16)
        # feat staged contiguously: [128(b*NT+it), (i_lo, d)] f32, then
        # transposed per-d on PE into feat_all [128(i_lo), (d, b, it)] f32
        feat_stage = singles.tile([128, 128 * D], f32)
        nc.sync.dma_start(
            feat_stage[:, :].rearrange("p (i d) -> p i d", i=128),
            feat_d.rearrange("b (t q) d -> (b t) q d", t=NT, q=128))
        feat_all = singles.tile([128, D * FW], f32)
        for d in range(D):
            psum_f = psE.tile([128, 128], f32, tag="pse")
            nc.tensor.transpose(
                psum_f[:, :],
                feat_stage[:, :].rearrange("p (i d) -> p d i", i=128)[:, d],
                ident32[:, :])
            nc.vector.tensor_copy(
                feat_all[:, d * FW:(d + 1) * FW], psum_f[:, :FW])
        # S_all: [128(j_lo), (q, b, jt)] f32
        S_all = singles.tile([128, max(NQ, 1) * NB * NT], f32)
        # A / mu: [128(i_lo), (d', b, it)] f32 ; nu: [128(j_lo), (b, jt, d)] f16
        A_all = singles.tile([128, D * FW], f32)
        mu = singles.tile([128, D * FW], f32)
        nu = singles.tile([128, FW * D], f16)

        # ---------- phase 1: per-batch streaming ----------
        for b in range(NB):
            g = b // GRP
            bg = b % GRP
            if bg == 0:
                psum_S = psS.tile([128, max(NQ, 1) * GRP * NT], f32, tag="psumS")
            xt = xpool.tile([128, NT * N], f32)
            nc.sync.dma_start(
                xt[:, :].rearrange("p (a c) -> p a c", a=NT),
                nl_d[b].rearrange("(a b) c -> b a c", a=NT))
            x16 = x16pool.tile([128, NT * N], f16)
            nc.gpsimd.tensor_copy(x16[:, :], xt[:, :])

            # hard-dim relu tiles
            rtiles = {}
            for h in hard_ds:
                rt = rpool.tile([128, NT * N], f16, tag=f"r{h}")
                nc.vector.tensor_scalar(
                    rt[:, :], x16[:, :], w4[h], -c4[h], Alu.mult, Alu.max)
                rtiles[h] = rt

            # reductions over i: stationary data tiles x ones -> psum cols
            for qi, (qk, qd) in enumerate(quants):
                src = x16 if qk == "sx" else rtiles[qd]
                for jt in range(NT):
                    col = qi * (GRP * NT) + bg * NT + jt
                    for it in range(NT):
                        nc.tensor.matmul(
                            psum_S[:, col:col + 1],
                            src[:, it * N + jt * 128: it * N + jt * 128 + 128],
                            ones16[:, :],
                            start=(it == 0), stop=(it == NT - 1))

            # transpose x16 -> psum (fp16), compare -> connT
            psum_x = psT.tile([128, NT * N], f16, tag="psumT")
            for it in range(NT):
                for jt in range(NT):
                    nc.tensor.transpose(
                        psum_x[:, jt * N + it * 128: jt * N + it * 128 + 128],
                        x16[:, it * N + jt * 128: it * N + jt * 128 + 128],
                        ident16[:, :])
            nc.scalar.sign(
                connT[:, b * (NT * N): (b + 1) * (NT * N)], psum_x[:, :])

            if bg == GRP - 1 and NQ > 0:
                # drain S psum group -> S_all (q, b, jt)
                nc.vector.tensor_copy(
                    S_all[:, :].rearrange(
                        "p (q b j) -> p q b j", q=max(NQ, 1), b=NB
                    )[:, :, g * GRP:(g + 1) * GRP, :],
                    psum_S[:, :].rearrange(
                        "p (q b j) -> p q b j", q=max(NQ, 1), b=GRP))

        # ---------- phase 2: build A ----------
        # A[:, d'-slice] = sum_d W1[d',d]*feat_d + const + sx_coef*SX + sh_coef*SH
        for dp in range(D):
            accv = A_all[:, dp * FW:(dp + 1) * FW]
            nc.vector.tensor_scalar(
                accv, feat_all[:, 0:FW], _f(W1[dp, 0]), a_const[dp],
                Alu.mult, Alu.add)
            for d in range(1, D):
                nc.vector.scalar_tensor_tensor(
                    accv, feat_all[:, d * FW:(d + 1) * FW], _f(W1[dp, d]), accv,
                    Alu.mult, Alu.add)
            for qi, (qk, qd) in enumerate(quants):
                coef = sx_coef[dp] if qk == "sx" else sh_coef[qd][dp]
                if coef != 0.0:
                    nc.vector.scalar_tensor_tensor(
                        accv, S_all[:, qi * FW:(qi + 1) * FW], coef, accv,
                        Alu.mult, Alu.add)

        # ---------- phase 3: iterations ----------
        def nu_fold():
            # nu[:, (b,jt,d_out)] = sum_d W2[d_out,d] * mu[:, (d,b,it)]
            nuv = nu[:, :].rearrange("p (b j d) -> p b j d", b=NB, j=NT)
            for do in range(D):
                accn = scr.tile([128, FW], f32, tag="nuacc")
                nc.vector.tensor_scalar(
                    accn[:, :], mu[:, 0 * FW:FW], _f(W2[do, 0]), None, Alu.mult)
                for d in range(1, D - 1):
                    nc.vector.scalar_tensor_tensor(
                        accn[:, :], mu[:, d * FW:(d + 1) * FW], _f(W2[do, d]),
                        accn[:, :], Alu.mult, Alu.add)
                nc.vector.scalar_tensor_tensor(
                    nuv[:, :, :, do].rearrange("p b j -> p (b j)"),
                    mu[:, (D - 1) * FW: D * FW], _f(W2[do, D - 1]),
                    accn[:, :], Alu.mult, Alu.add)

        if n_iters == 0:
            nc.vector.memset(mu[:, :], 0.0)
        else:
            # mu_1 = relu(A)
            nc.vector.tensor_scalar(mu[:, :], A_all[:, :], 0.0, None, Alu.max)
            for t in range(1, n_iters):
                nu_fold()
                for g in range(NG):
                    psum_mu = psMU.tile([128, GRP * NT * D], f32, tag="psumMU")
                    for bg in range(GRP):
                        b = g * GRP + bg
                        for it in range(NT):
                            slot = (bg * NT + it) * D
                            for jt in range(NT):
                                nc.tensor.matmul(
                                    psum_mu[:, slot:slot + D],
                                    connT[:, b * (NT * N) + jt * N + it * 128:
                                          b * (NT * N) + jt * N + it * 128 + 128],
                                    nu[:, (b * NT + jt) * D:(b * NT + jt) * D + D],
                                    start=(jt == 0), stop=(jt == NT - 1))
                    # mu_g = relu(psum + A_g)
                    tmp = scr.tile([128, GRP * NT * D], f32, tag="updtmp")
                    Ag = A_all[:, :].rearrange(
                        "p (d b i) -> p b i d", d=D, b=NB)[:, g * GRP:(g + 1) * GRP]
                    nc.vector.scalar_tensor_tensor(
                        tmp[:, :].rearrange("p (b i d) -> p b i d", b=GRP, i=NT),
                        psum_mu[:, :].rearrange("p (b i d) -> p b i d", b=GRP, i=NT),
                        0.0, Ag, Alu.add, Alu.add)
                    mug = mu[:, :].rearrange(
                        "p (d b i) -> p b i d", d=D, b=NB)[:, g * GRP:(g + 1) * GRP]
                    nc.vector.tensor_scalar(
                        mug,
                        tmp[:, :].rearrange("p (b i d) -> p b i d", b=GRP, i=NT),
                        0.0, None, Alu.max)

        # ---------- phase 4: epilogue ----------
        # ms[c=(b,it), d'] = sum_{i_lo} mu[i_lo, (d', c)]
        psum_ms = psE.tile([128, 128], f32, tag="pse")
        for dp in range(D):
            nc.tensor.matmul(
                psum_ms[:FW, dp:dp + 1],
                mu[:, dp * FW:(dp + 1) * FW],
                ones32[:, :], start=True, stop=True)
        ms_sb = scr.tile([128, D], f32, tag="mssb")
        nc.vector.tensor_copy(ms_sb[:FW, :], psum_ms[:FW, :D])
        # msum[b, d'] = sum over the NT halves
        psum_msum = psE.tile([128, 128], f32, tag="pse")
        nc.tensor.matmul(psum_msum[:NB, :D], pairmask[:FW, :NB], ms_sb[:FW, :D],
                         start=True, stop=True)
        msum_sb = scr.tile([NB, D], f32, tag="msumsb")
        nc.vector.tensor_copy(msum_sb[:, :], psum_msum[:NB, :D])
        # msF[c, d'] = msum[c//NT, d']  (replicated back to (b,it) partitions)
        psum_msF = psE.tile([128, 128], f32, tag="pse")
        nc.tensor.matmul(psum_msF[:FW, :D], pmkT[:NB, :FW], msum_sb[:NB, :D],
                         start=True, stop=True)
        msF = scr.tile([128, D], f32, tag="msF")
        nc.vector.tensor_copy(msF[:FW, :], psum_msF[:FW, :D])
        # t6[c, d''] = sum_d' W6[d'',d'] msF[c,d'] + b6[d'']  -> relu -> * w5a
        t6 = scr.tile([128, D], f32, tag="t6")
        for dpp in range(D):
            nc.vector.tensor_scalar(
                t6[:FW, dpp:dpp + 1], msF[:FW, 0:1], _f(W6[dpp, 0]),
                _f(b6[dpp]), Alu.mult, Alu.add)
            for dp in range(1, D):
                nc.vector.scalar_tensor_tensor(
                    t6[:FW, dpp:dpp + 1], msF[:FW, dp:dp + 1], _f(W6[dpp, dp]),
                    t6[:FW, dpp:dpp + 1], Alu.mult, Alu.add)
        r6 = scr.tile([128, D], f32, tag="r6")
        nc.vector.tensor_scalar(r6[:FW, :], t6[:FW, :], 0.0, None, Alu.max)
        g6 = scr.tile([128, 1], f32, tag="g6")
        nc.vector.tensor_scalar(
            g6[:FW, :], r6[:FW, 0:1], _f(W5[0, 0]), _f(b5[0]), Alu.mult, Alu.add)
        for dpp in range(1, D):
            nc.vector.scalar_tensor_tensor(
                g6[:FW, :], r6[:FW, dpp:dpp + 1], _f(W5[0, dpp]), g6[:FW, :],
                Alu.mult, Alu.add)

        # term7 path: opre[i_lo, c] = sum_d'' w5b[d''] relu(t7_d'')
        opre = scr.tile([128, FW], f32, tag="opre")
        t7 = scr.tile([128, FW], f32, tag="t7")
        r7 = scr.tile([128, FW], f32, tag="r7")
        for dpp in range(D):
            nc.vector.tensor_scalar(
                t7[:, :], mu[:, 0 * FW:FW], _f(W7[dpp, 0]), _f(b7[dpp]),
                Alu.mult, Alu.add)
            for d in range(1, D):
                nc.vector.scalar_tensor_tensor(
                    t7[:, :], mu[:, d * FW:(d + 1) * FW], _f(W7[dpp, d]),
                    t7[:, :], Alu.mult, Alu.add)
            if dpp == 0:
                nc.vector.tensor_scalar(
                    opre[:, :], t7[:, :], 0.0, _f(W5[0, D + dpp]),
                    Alu.max, Alu.mult)
            else:
                nc.vector.tensor_scalar(
                    r7[:, :], t7[:, :], 0.0, _f(W5[0, D + dpp]),
                    Alu.max, Alu.mult)
                nc.vector.tensor_add(opre[:, :], opre[:, :], r7[:, :])

        # transpose opre -> [c=(b,it), i_lo], add g6 per-partition, DMA out
        psum_o = psE.tile([128, 128], f32, tag="pse")
        nc.tensor.transpose(psum_o[:FW, :128], opre[:, :FW], ident32[:, :])
        outT = scr.tile([128, 128], f32, tag="outT")
        nc.vector.tensor_scalar(
            outT[:FW, :], psum_o[:FW, :], g6[:FW, 0:1], None, Alu.add)
        nc.sync.dma_start(
            out_d.rearrange("a (b c) -> (a b) c", b=NT, c=128),
            outT[:FW, :128])

    nc.compile()
    return nc


def _get_nc(B_shard, n_iters, w, xmin, xmax):
    key = (B_shard, n_iters, xmin, xmax,
           tuple(np.asarray(w[k]).tobytes() for k in sorted(w)))
    if key not in _CACHE:
        _CACHE[key] = _build(B_shard, n_iters, w, xmin, xmax)
    return _CACHE[key]


def kernel(features, nodeLocations, W1, b1, W2, b2, W3, b3, W4, b4,
           W5, b5, W6, b6, W7, b7, embedding_iteration):
    features = np.ascontiguousarray(np.asarray(features, dtype=np.float32))
    nodeLocations = np.ascontiguousarray(
        np.asarray(nodeLocations, dtype=np.float32))
    B = features.shape[0]
    assert B % N_CORES == 0
    Bs = B // N_CORES
    n_iters = int(np.asarray(embedding_iteration))
    w = dict(W1=np.asarray(W1), b1=np.asarray(b1), W2=np.asarray(W2),
             b2=np.asarray(b2), W3=np.asarray(W3), b3=np.asarray(b3),
             W4=np.asarray(W4), b4=np.asarray(b4), W5=np.asarray(W5),
             b5=np.asarray(b5), W6=np.asarray(W6), b6=np.asarray(b6),
             W7=np.asarray(W7), b7=np.asarray(b7))
    xmin = float(nodeLocations.min())
    xmax = float(nodeLocations.max())

    nc = _get_nc(Bs, n_iters, w, xmin, xmax)

    from concourse.bass_utils import run_bass_kernel_spmd
    in_maps = []
    for c in range(N_CORES):
        in_maps.append({
            "nl": nodeLocations[c * Bs:(c + 1) * Bs],
            "feat": features[c * Bs:(c + 1) * Bs],
        })
    res = run_bass_kernel_spmd(
        nc, in_maps, core_ids=list(range(N_CORES)),
        trace=bool(int(os.environ.get("KBENCH_TRACE", "0"))))
    out = np.concatenate([r["out"] for r in res.results], axis=0)
    if res.exec_time_ns is not None:
        kernel.last_exec_time_ns = res.exec_time_ns
    kernel.last_results = res
    return out


kernel.last_exec_time_ns = None
kernel.last_results = None


# revision 12
# speedup vs baseline: 1.4970x; 1.4970x over previous
"""Trainium2 Bass kernel for the DQN GNN message-passing module.

Contract: kernel(**inputs) takes the FULL unsharded inputs (as produced by
setup_inputs) and returns the FULL [B, N] output. Internally the batch is
sharded across 8 NeuronCores (pure data parallel), a Bass/Tile kernel is
compiled and run via run_bass_kernel_spmd, and the per-core outputs are
concatenated.

Math (per batch b, N=256 nodes, D=5):
  term1 = feat @ W1.T + b1
  S[j,d] = sum_i relu(W4[d]*x[i,j] + b4[d])        (x = nodeLocations[b])
  term3 = S @ W3.T + b3
  conn  = (x > 0)
  mu_0 = 0;  mu_{t+1} = relu(A + (conn @ (mu_t @ W2.T)))   A = term1+term3+b2
  out = relu([W6(sum_i mu) | W7 mu]) @ W5.T + b5

Key tricks:
  * per-d case analysis of relu(w*x+c) over the actual input range
    [xmin,xmax]: always-nonpositive dims contribute 0, always-nonnegative
    dims are linear in sum_i x, only "knee inside range" dims need an
    elementwise pass (max(w*x,-c), the +c folded into constants).
  * all sum-over-i reductions are PE matmuls with the DATA as the
    128x128 stationary operand and a [128,1] ones moving operand, so the
    result lands with the free index on PSUM partitions.
  * conn transposed on PE (fp16), compare fused into the PSUM drain.
"""

import os
import numpy as np
from contextlib import ExitStack

N_CORES = 8
N = 256
D = 5

_CACHE = {}


def _f(x):
    return float(np.asarray(x))


def _build(B_shard, n_iters, w, xmin, xmax):
    import concourse.bass as bass
    import concourse.bacc as bacc
    import concourse.tile as tile
    import concourse.mybir as mybir

    Alu = mybir.AluOpType
    f32 = mybir.dt.float32
    f16 = mybir.dt.float16

    W1, b1 = w["W1"], w["b1"]
    W2, b2 = w["W2"], w["b2"]
    W3, b3 = w["W3"], w["b3"]
    W4, b4 = w["W4"], w["b4"]
    W5, b5 = w["W5"], w["b5"]
    W6, b6 = w["W6"], w["b6"]
    W7, b7 = w["W7"], w["b7"]

    # --- case analysis for S[j,d] = sum_i relu(w_d x + c_d), x in [xmin,xmax]
    w4 = [_f(W4[d, 0]) for d in range(D)]
    c4 = [_f(b4[d]) for d in range(D)]
    # kind per d: 'Z' (relu==0), 'L' (relu==w x+c), 'H' (needs elementwise)
    kinds = []
    for d in range(D):
        lo = w4[d] * xmin + c4[d]
        hi = w4[d] * xmax + c4[d]
        if max(lo, hi) <= 0.0:
            kinds.append("Z")
        elif min(lo, hi) >= 0.0:
            kinds.append("L")
        else:
            kinds.append("H")
    need_sx = any(k == "L" for k in kinds)
    hard_ds = [d for d in range(D) if kinds[d] == "H"]
    # S_d = w_d*SX + 256*c_d            (L)
    #     = SH_d + 256*c_d              (H), SH_d = sum_i max(w_d x, -c_d)
    #     = 0                           (Z)
    # term3[n,d'] = sum_d W3[d',d] S_d[n] + b3[d']
    # A[n,d'] = term1[n,d'] + term3[n,d'] + b2[d']
    # constant part of A (indep of data, except through S constants):
    a_const = [0.0] * D
    sx_coef = [0.0] * D  # coefficient of SX in A[., d']
    sh_coef = {h: [0.0] * D for h in hard_ds}  # coefficient of SH_h
    for dp in range(D):
        a_const[dp] = _f(b1[dp]) + _f(b2[dp]) + _f(b3[dp])
        for d in range(D):
            if kinds[d] == "L":
                a_const[dp] += _f(W3[dp, d]) * (N * c4[d])
                sx_coef[dp] += _f(W3[dp, d]) * w4[d]
            elif kinds[d] == "H":
                a_const[dp] += _f(W3[dp, d]) * (N * c4[d])
                sh_coef[d][dp] = _f(W3[dp, d])

    # S quantity slots in S_all: q=0 -> SX (if needed), then hard dims
    quants = []
    if need_sx:
        quants.append(("sx", None))
    for h in hard_ds:
        quants.append(("h", h))
    NQ = len(quants)

    nc = bacc.Bacc("TRN2", target_bir_lowering=False, debug=False)
    nl_d = nc.dram_tensor("nl", [B_shard, N, N], f32, kind="ExternalInput").ap()
    feat_d = nc.dram_tensor("feat", [B_shard, N, D], f32, kind="ExternalInput").ap()
    out_d = nc.dram_tensor("out", [B_shard, N], f32, kind="ExternalOutput").ap()

    NB = B_shard          # batches per core
    NT = N // 128         # partition tiles per node dim (2)
    FW = NB * NT          # 128 for NB=64 (b,it)-columns
    GRP = 16              # batches per PSUM S-group
    NG = (NB + GRP - 1) // GRP

    assert NB % GRP == 0 and FW <= 128

    with tile.TileContext(nc) as tc, ExitStack() as ctx:
        singles = ctx.enter_context(tc.tile_pool(name="singles", bufs=1))
        xpool = ctx.enter_context(tc.tile_pool(name="xpool", bufs=8))
        x16pool = ctx.enter_context(tc.tile_pool(name="x16pool", bufs=3))
        rpool = ctx.enter_context(tc.tile_pool(name="rpool", bufs=3))
        psT = ctx.enter_context(tc.tile_pool(name="psT", bufs=3, space="PSUM"))
        psS = ctx.enter_context(tc.tile_pool(name="psS", bufs=2, space="PSUM"))
        psMU = ctx.enter_context(tc.tile_pool(name="psMU", bufs=2, space="PSUM"))
        psE = ctx.enter_context(tc.tile_pool(name="psE", bufs=1, space="PSUM"))
        scr = ctx.enter_context(tc.tile_pool(name="scr", bufs=2))

        # ---------- constants ----------
        ident16 = singles.tile([128, 128], f16)
        ident32 = singles.tile([128, 128], f32)
        from concourse import masks as _masks
        _masks.make_identity(nc, ident16[:, :])
        _masks.make_identity(nc, ident32[:, :])
        ones16 = singles.tile([128, 1], f16)
        nc.vector.memset(ones16[:, :], 1.0)
        ones32 = singles.tile([128, 1], f32)
        nc.vector.memset(ones32[:, :], 1.0)
        # pairmask[p, c] = 1 iff p//NT == c  (shape [128, NB])  -- built via
        # affine iota: keep where 0 <= p - NT*c <= NT-1
        pairmask = singles.tile([128, NB], f32)
        nc.gpsimd.memset(pairmask[:, :], 1.0)
        nc.gpsimd.affine_select(
            out=pairmask[:, :], in_=pairmask[:, :],
            pattern=[[-NT, NB]], compare_op=Alu.is_ge, fill=0.0,
            base=0, channel_multiplier=1)
        nc.gpsimd.affine_select(
            out=pairmask[:, :], in_=pairmask[:, :],
            pattern=[[NT, NB]], compare_op=Alu.is_ge, fill=0.0,
            base=NT - 1, channel_multiplier=-1)
        # pmkT = pairmask.T  [NB, 128]
        ps_tmp = psE.tile([128, 128], f32, tag="pse")
        nc.tensor.transpose(ps_tmp[:NB, :128], pairmask[:, :NB], ident32[:, :])
        pmkT = singles.tile([NB, 128], f32)
        nc.vector.tensor_copy(pmkT[:, :], ps_tmp[:NB, :128])

        # ---------- per-core big buffers ----------
        # connT_all: [128(j_lo), (b, jt, i)] fp16
        connT = singles.tile([128, NB * NT * N], f16)
        # feat staged contiguously: [128(b*NT+it), (i_lo, d)] f32, then
        # transposed per-d on PE into feat_all [128(i_lo), (d, b, it)] f32
        feat_stage = singles.tile([128, 128 * D], f32)
        nc.sync.dma_start(
            feat_stage[:, :].rearrange("p (i d) -> p i d", i=128),
            feat_d.rearrange("b (t q) d -> (b t) q d", t=NT, q=128))
        feat_all = singles.tile([128, D * FW], f32)
        for d in range(D):
            psum_f = psE.tile([128, 128], f32, tag="pse")
            nc.tensor.transpose(
                psum_f[:, :],
                feat_stage[:, :].rearrange("p (i d) -> p d i", i=128)[:, d],
                ident32[:, :])
            nc.vector.tensor_copy(
                feat_all[:, d * FW:(d + 1) * FW], psum_f[:, :FW])
        # S_all: [128(j_lo), (q, b, jt)] f32
        S_all = singles.tile([128, max(NQ, 1) * NB * NT], f32)
        # A / mu: [128(i_lo), (d', b, it)] f32 ; nu: [128(j_lo), (b, jt, d)] f16
        A_all = singles.tile([128, D * FW], f32)
        mu = singles.tile([128, D * FW], f32)
        nu = singles.tile([128, FW * D], f16)

        # ---------- phase 1: per-batch streaming ----------
        for b in range(NB):
            g = b // GRP
            bg = b % GRP
            if bg == 0:
                psum_S = psS.tile([128, max(NQ, 1) * GRP * NT], f32, tag="psumS")
            xt = xpool.tile([128, NT * N], f32)
            nc.sync.dma_start(
                xt[:, :].rearrange("p (a c) -> p a c", a=NT),
                nl_d[b].rearrange("(a b) c -> b a c", a=NT))
            x16 = x16pool.tile([128, NT * N], f16)
            nc.vector.tensor_copy(x16[:, :], xt[:, :])

            # hard-dim relu tiles
            rtiles = {}
            for h in hard_ds:
                rt = rpool.tile([128, NT * N], f16, tag=f"r{h}")
                nc.vector.tensor_scalar(
                    rt[:, :], x16[:, :], w4[h], -c4[h], Alu.mult, Alu.max)
                rtiles[h] = rt

            # reductions over i: stationary data tiles x ones -> psum cols
            for qi, (qk, qd) in enumerate(quants):
                src = x16 if qk == "sx" else rtiles[qd]
                for jt in range(NT):
                    col = qi * (GRP * NT) + bg * NT + jt
                    for it in range(NT):
                        nc.tensor.matmul(
                            psum_S[:, col:col + 1],
                            src[:, it * N + jt * 128: it * N + jt * 128 + 128],
                            ones16[:, :],
                            start=(it == 0), stop=(it == NT - 1))

            # transpose x16 -> psum (fp16), compare -> connT
            psum_x = psT.tile([128, NT * N], f16, tag="psumT")
            for it in range(NT):
                for jt in range(NT):
                    nc.tensor.transpose(
                        psum_x[:, jt * N + it * 128: jt * N + it * 128 + 128],
                        x16[:, it * N + jt * 128: it * N + jt * 128 + 128],
                        ident16[:, :])
            nc.scalar.sign(
                connT[:, b * (NT * N): (b + 1) * (NT * N)], psum_x[:, :])

            if bg == GRP - 1 and NQ > 0:
                # drain S psum group -> S_all (q, b, jt)
                nc.vector.tensor_copy(
                    S_all[:, :].rearrange(
                        "p (q b j) -> p q b j", q=max(NQ, 1), b=NB
                    )[:, :, g * GRP:(g + 1) * GRP, :],
                    psum_S[:, :].rearrange(
                        "p (q b j) -> p q b j", q=max(NQ, 1), b=GRP))

        # ---------- phase 2: build A ----------
        # A[:, d'-slice] = sum_d W1[d',d]*feat_d + const + sx_coef*SX + sh_coef*SH
        for dp in range(D):
            accv = A_all[:, dp * FW:(dp + 1) * FW]
            nc.vector.tensor_scalar(
                accv, feat_all[:, 0:FW], _f(W1[dp, 0]), a_const[dp],
                Alu.mult, Alu.add)
            for d in range(1, D):
                nc.vector.scalar_tensor_tensor(
                    accv, feat_all[:, d * FW:(d + 1) * FW], _f(W1[dp, d]), accv,
                    Alu.mult, Alu.add)
            for qi, (qk, qd) in enumerate(quants):
                coef = sx_coef[dp] if qk == "sx" else sh_coef[qd][dp]
                if coef != 0.0:
                    nc.vector.scalar_tensor_tensor(
                        accv, S_all[:, qi * FW:(qi + 1) * FW], coef, accv,
                        Alu.mult, Alu.add)

        # ---------- phase 3: iterations ----------
        def nu_fold():
            # nu[:, (b,jt,d_out)] = sum_d W2[d_out,d] * mu[:, (d,b,it)]
            nuv = nu[:, :].rearrange("p (b j d) -> p b j d", b=NB, j=NT)
            for do in range(D):
                accn = scr.tile([128, FW], f32, tag="nuacc")
                nc.vector.tensor_scalar(
                    accn[:, :], mu[:, 0 * FW:FW], _f(W2[do, 0]), None, Alu.mult)
                for d in range(1, D - 1):
                    nc.vector.scalar_tensor_tensor(
                        accn[:, :], mu[:, d * FW:(d + 1) * FW], _f(W2[do, d]),
                        accn[:, :], Alu.mult, Alu.add)
                nc.vector.scalar_tensor_tensor(
                    nuv[:, :, :, do].rearrange("p b j -> p (b j)"),
                    mu[:, (D - 1) * FW: D * FW], _f(W2[do, D - 1]),
                    accn[:, :], Alu.mult, Alu.add)

        if n_iters == 0:
            nc.vector.memset(mu[:, :], 0.0)
        else:
            # mu_1 = relu(A)
            nc.vector.tensor_scalar(mu[:, :], A_all[:, :], 0.0, None, Alu.max)
            for t in range(1, n_iters):
                nu_fold()
                for g in range(NG):
                    psum_mu = psMU.tile([128, GRP * NT * D], f32, tag="psumMU")
                    for bg in range(GRP):
                        b = g * GRP + bg
                        for it in range(NT):
                            slot = (bg * NT + it) * D
                            for jt in range(NT):
                                nc.tensor.matmul(
                                    psum_mu[:, slot:slot + D],
                                    connT[:, b * (NT * N) + jt * N + it * 128:
                                          b * (NT * N) + jt * N + it * 128 + 128],
                                    nu[:, (b * NT + jt) * D:(b * NT + jt) * D + D],
                                    start=(jt == 0), stop=(jt == NT - 1))
                    # mu_g = relu(psum + A_g)
                    tmp = scr.tile([128, GRP * NT * D], f32, tag="updtmp")
                    Ag = A_all[:, :].rearrange(
                        "p (d b i) -> p b i d", d=D, b=NB)[:, g * GRP:(g + 1) * GRP]
                    nc.vector.scalar_tensor_tensor(
                        tmp[:, :].rearrange("p (b i d) -> p b i d", b=GRP, i=NT),
                        psum_mu[:, :].rearrange("p (b i d) -> p b i d", b=GRP, i=NT),
                        0.0, Ag, Alu.add, Alu.add)
                    mug = mu[:, :].rearrange(
                        "p (d b i) -> p b i d", d=D, b=NB)[:, g * GRP:(g + 1) * GRP]
                    nc.vector.tensor_scalar(
                        mug,
                        tmp[:, :].rearrange("p (b i d) -> p b i d", b=GRP, i=NT),
                        0.0, None, Alu.max)

        # ---------- phase 4: epilogue ----------
        # ms[c=(b,it), d'] = sum_{i_lo} mu[i_lo, (d', c)]
        psum_ms = psE.tile([128, 128], f32, tag="pse")
        for dp in range(D):
            nc.tensor.matmul(
                psum_ms[:FW, dp:dp + 1],
                mu[:, dp * FW:(dp + 1) * FW],
                ones32[:, :], start=True, stop=True)
        ms_sb = scr.tile([128, D], f32, tag="mssb")
        nc.vector.tensor_copy(ms_sb[:FW, :], psum_ms[:FW, :D])
        # msum[b, d'] = sum over the NT halves
        psum_msum = psE.tile([128, 128], f32, tag="pse")
        nc.tensor.matmul(psum_msum[:NB, :D], pairmask[:FW, :NB], ms_sb[:FW, :D],
                         start=True, stop=True)
        msum_sb = scr.tile([NB, D], f32, tag="msumsb")
        nc.vector.tensor_copy(msum_sb[:, :], psum_msum[:NB, :D])
        # msF[c, d'] = msum[c//NT, d']  (replicated back to (b,it) partitions)
        psum_msF = psE.tile([128, 128], f32, tag="pse")
        nc.tensor.matmul(psum_msF[:FW, :D], pmkT[:NB, :FW], msum_sb[:NB, :D],
                         start=True, stop=True)
        msF = scr.tile([128, D], f32, tag="msF")
        nc.vector.tensor_copy(msF[:FW, :], psum_msF[:FW, :D])
        # t6[c, d''] = sum_d' W6[d'',d'] msF[c,d'] + b6[d'']  -> relu -> * w5a
        t6 = scr.tile([128, D], f32, tag="t6")
        for dpp in range(D):
            nc.vector.tensor_scalar(
                t6[:FW, dpp:dpp + 1], msF[:FW, 0:1], _f(W6[dpp, 0]),
                _f(b6[dpp]), Alu.mult, Alu.add)
            for dp in range(1, D):
                nc.vector.scalar_tensor_tensor(
                    t6[:FW, dpp:dpp + 1], msF[:FW, dp:dp + 1], _f(W6[dpp, dp]),
                    t6[:FW, dpp:dpp + 1], Alu.mult, Alu.add)
        r6 = scr.tile([128, D], f32, tag="r6")
        nc.vector.tensor_scalar(r6[:FW, :], t6[:FW, :], 0.0, None, Alu.max)
        g6 = scr.tile([128, 1], f32, tag="g6")
        nc.vector.tensor_scalar(
            g6[:FW, :], r6[:FW, 0:1], _f(W5[0, 0]), _f(b5[0]), Alu.mult, Alu.add)
        for dpp in range(1, D):
            nc.vector.scalar_tensor_tensor(
                g6[:FW, :], r6[:FW, dpp:dpp + 1], _f(W5[0, dpp]), g6[:FW, :],
                Alu.mult, Alu.add)

        # term7 path: opre[i_lo, c] = sum_d'' w5b[d''] relu(t7_d'')
        opre = scr.tile([128, FW], f32, tag="opre")
        t7 = scr.tile([128, FW], f32, tag="t7")
        r7 = scr.tile([128, FW], f32, tag="r7")
        for dpp in range(D):
            nc.vector.tensor_scalar(
                t7[:, :], mu[:, 0 * FW:FW], _f(W7[dpp, 0]), _f(b7[dpp]),
                Alu.mult, Alu.add)
            for d in range(1, D):
                nc.vector.scalar_tensor_tensor(
                    t7[:, :], mu[:, d * FW:(d + 1) * FW], _f(W7[dpp, d]),
                    t7[:, :], Alu.mult, Alu.add)
            if dpp == 0:
                nc.vector.tensor_scalar(
                    opre[:, :], t7[:, :], 0.0, _f(W5[0, D + dpp]),
                    Alu.max, Alu.mult)
            else:
                nc.vector.tensor_scalar(
                    r7[:, :], t7[:, :], 0.0, _f(W5[0, D + dpp]),
                    Alu.max, Alu.mult)
                nc.vector.tensor_add(opre[:, :], opre[:, :], r7[:, :])

        # transpose opre -> [c=(b,it), i_lo], add g6 per-partition, DMA out
        psum_o = psE.tile([128, 128], f32, tag="pse")
        nc.tensor.transpose(psum_o[:FW, :128], opre[:, :FW], ident32[:, :])
        outT = scr.tile([128, 128], f32, tag="outT")
        nc.vector.tensor_scalar(
            outT[:FW, :], psum_o[:FW, :], g6[:FW, 0:1], None, Alu.add)
        nc.sync.dma_start(
            out_d.rearrange("a (b c) -> (a b) c", b=NT, c=128),
            outT[:FW, :128])

    nc.compile()
    return nc


def _get_nc(B_shard, n_iters, w, xmin, xmax):
    key = (B_shard, n_iters, xmin, xmax,
           tuple(np.asarray(w[k]).tobytes() for k in sorted(w)))
    if key not in _CACHE:
        _CACHE[key] = _build(B_shard, n_iters, w, xmin, xmax)
    return _CACHE[key]


def kernel(features, nodeLocations, W1, b1, W2, b2, W3, b3, W4, b4,
           W5, b5, W6, b6, W7, b7, embedding_iteration):
    features = np.ascontiguousarray(np.asarray(features, dtype=np.float32))
    nodeLocations = np.ascontiguousarray(
        np.asarray(nodeLocations, dtype=np.float32))
    B = features.shape[0]
    assert B % N_CORES == 0
    Bs = B // N_CORES
    n_iters = int(np.asarray(embedding_iteration))
    w = dict(W1=np.asarray(W1), b1=np.asarray(b1), W2=np.asarray(W2),
             b2=np.asarray(b2), W3=np.asarray(W3), b3=np.asarray(b3),
             W4=np.asarray(W4), b4=np.asarray(b4), W5=np.asarray(W5),
             b5=np.asarray(b5), W6=np.asarray(W6), b6=np.asarray(b6),
             W7=np.asarray(W7), b7=np.asarray(b7))
    xmin = float(nodeLocations.min())
    xmax = float(nodeLocations.max())

    nc = _get_nc(Bs, n_iters, w, xmin, xmax)

    from concourse.bass_utils import run_bass_kernel_spmd
    in_maps = []
    for c in range(N_CORES):
        in_maps.append({
            "nl": nodeLocations[c * Bs:(c + 1) * Bs],
            "feat": features[c * Bs:(c + 1) * Bs],
        })
    res = run_bass_kernel_spmd(
        nc, in_maps, core_ids=list(range(N_CORES)),
        trace=bool(int(os.environ.get("KBENCH_TRACE", "0"))))
    out = np.concatenate([r["out"] for r in res.results], axis=0)
    if res.exec_time_ns is not None:
        kernel.last_exec_time_ns = res.exec_time_ns
    kernel.last_results = res
    return out


kernel.last_exec_time_ns = None
kernel.last_results = None
